# revision 5
# baseline (speedup 1.0000x reference)
"""Trainium2 Bass kernel for nn_GRUDecoder (GNN message-passing GRU decoder).

Sharding: data-parallel over batch B=16 across 8 cores (2 batch elems/core),
all parameters replicated. Forward only -> no collectives.

Math (per step, per batch elem), restructured for TRN2:
  senders   = rel_send @ hidden            (one-hot gather)
  receivers = rel_rec  @ hidden
  msg1 = relu([senders, receivers] @ W1.T + b1)
       = relu(rel_send @ (hidden @ W1s.T) + rel_rec @ (hidden @ W1r.T) + b1)
    -> apply W1 at NODE level (64 rows) first, then gather to 4096 edges via
       one-hot matmuls on the PE (fuses gather + add into PSUM accumulation).
  msg2 = relu(msg1 @ W2.T + b2)            (feature-major matmul, edges stream)
  agg  = (msg2 * type_sum).T @ rel_rec     (fold type_sum into rel_rec host-side;
                                            edge-major matmul after DMA transpose)
  GRU + 3-layer output MLP: small feature-major matmuls, batched over (b, n).

Everything stays resident in SBUF across the 39 sequential steps.
"""

import sys

for _p in ("/opt/trn_rl_repo", "/opt/pypackages"):
    if _p not in sys.path:
        sys.path.append(_p)

import numpy as np
import ml_dtypes

import concourse.bass as bass
import concourse.mybir as mybir
import concourse.tile as tile
from concourse import bacc
from concourse.bass_utils import run_bass_kernel_spmd

# Problem constants (hardcoded per contract)
B, T, N, DIN = 16, 40, 64, 64
S = T - 1            # 39 scan steps
H = 256
DOUT = 64
E = N * (N - 1)      # 4032
EG = 4096            # edges padded to 4096 (pad rows contribute 0 via W_agg)
NCORES = 8
BPC = B // NCORES    # 2 batch elems per core
BN = BPC * N         # 128 = (b, n) packed partition dim
TN = S * N           # 2496

F32 = mybir.dt.float32
F32R = mybir.dt.float32r
BF16 = mybir.dt.bfloat16
F16 = mybir.dt.float16
EDT = F16            # edge-pipeline matmul dtype
GDT = F16            # gru/output-mlp matmul dtype
NP_EDT = np.float16
NP_GDT = np.float16

AF = mybir.ActivationFunctionType
ALU = mybir.AluOpType


def _chunk2(w):
    """[256, F] -> [128, 2, F] (partition, chunk, free)."""
    f = w.shape[1]
    return np.ascontiguousarray(w.reshape(2, 128, f).transpose(1, 0, 2))


def _chunk22(w):
    """[256, 256] -> [128, kc, mc, 128]."""
    return np.ascontiguousarray(
        w.reshape(2, 128, 2, 128).transpose(1, 0, 2, 3))


def build_program():
    nc = bacc.Bacc("TRN2", target_bir_lowering=False, debug=False)

    def din(name, shape, dtype):
        return nc.dram_tensor(name, list(shape), dtype, kind="ExternalInput").ap()

    xT = din("xT", [64, BPC, TN], F32)            # din-major inputs
    wxT = din("wxT", [64, 768], F32)              # [Wir;Wii;Win].T
    bx = din("bx", [128, 6], F32)                 # x-side biases, chunked
    rsT = din("rsT", [128, EG], EDT)              # rel_send.T stacked twice
    rrT = din("rrT", [128, EG], EDT)              # rel_rec.T stacked twice
    w1sT = din("w1sT", [128, 2, 256], EDT)
    w1rT = din("w1rT", [128, 2, 256], EDT)
    w2T = din("w2T", [128, 2, 2, 128], EDT)
    wagg = din("wagg", [128, BPC, 32, 64], EDT)   # ts-scaled rel_rec, permuted
    whT = din("whT", [128, 3, 2, 2, 128], GDT)    # gru_h{r,i,n}_w.T
    ident = din("ident", [128, 128], GDT)
    wo1T = din("wo1T", [128, 2, 2, 128], GDT)
    wo2T = din("wo2T", [128, 2, 2, 128], GDT)
    wo3T = din("wo3T", [128, 2, 64], GDT)
    b1c = din("b1c", [128, 2], F32)
    b2c = din("b2c", [128, 2], F32)
    bh = din("bh", [128, 3, 2], F32)
    bo12 = din("bo12", [128, 2, 2], F32)
    bo3c = din("bo3c", [64, 1], F32)

    predT = nc.dram_tensor("predT", [S, DOUT, BPC, N], F32,
                           kind="ExternalOutput").ap()

    with tile.TileContext(nc) as tc:
        with (
            tc.tile_pool(name="const", bufs=1) as const,
            tc.tile_pool(name="mpsum", bufs=4, space="PSUM") as mpsum,
            tc.tile_pool(name="spsum", bufs=3, space="PSUM") as spsum,
            tc.tile_pool(name="aggps", bufs=1, space="PSUM") as aggps,
            tc.tile_pool(name="msgs", bufs=4) as msgs,
            tc.tile_pool(name="msgs2", bufs=2) as msgs2,
            tc.tile_pool(name="work", bufs=2) as work,
            tc.tile_pool(name="hidp", bufs=2) as hidp,
            tc.tile_pool(name="xdram", bufs=1, space="DRAM") as xdram,
        ):
            # ---- load constants into SBUF ----
            def cload(ap_in, shape, dtype, tag):
                t = const.tile(list(shape), dtype, tag=tag)
                nc.sync.dma_start(t[:], ap_in)
                return t

            rsT_s = cload(rsT, [128, EG], EDT, "rsT")
            rrT_s = cload(rrT, [128, EG], EDT, "rrT")
            w1sT_s = cload(w1sT, [128, 2, 256], EDT, "w1sT")
            w1rT_s = cload(w1rT, [128, 2, 256], EDT, "w1rT")
            w2T_s = cload(w2T, [128, 2, 2, 128], EDT, "w2T")
            wagg_s = cload(wagg, [128, BPC, 32, 64], EDT, "wagg")
            whT_s = cload(whT, [128, 3, 2, 2, 128], GDT, "whT")
            ident_s = cload(ident, [128, 128], GDT, "ident")
            wo1T_s = cload(wo1T, [128, 2, 2, 128], GDT, "wo1T")
            wo2T_s = cload(wo2T, [128, 2, 2, 128], GDT, "wo2T")
            wo3T_s = cload(wo3T, [128, 2, 64], GDT, "wo3T")
            b1c_s = cload(b1c, [128, 2], F32, "b1c")
            b2c_s = cload(b2c, [128, 2], F32, "b2c")
            bh_s = cload(bh, [128, 3, 2], F32, "bh")
            bo12_s = cload(bo12, [128, 2, 2], F32, "bo12")
            bo3c_s = cload(bo3c, [64, 1], F32, "bo3c")

            # X-gate DRAM scratch: [S, gate, chunk, go_part, (b, n)]
            XD = xdram.tile([S, 3, 2, 128, BN], GDT)

            # ---- phase X: precompute x-side GRU gate inputs for all t ----
            with tc.tile_pool(name="xphase", bufs=1) as xp:
                xT_f = xp.tile([64, BPC, TN], F32, tag="xTf")
                nc.sync.dma_start(xT_f[:], xT)
                xT_s = xp.tile([64, BPC, TN], F32R, tag="xT")
                nc.vector.tensor_copy(xT_s[:], xT_f[:])
                wxT_f = xp.tile([64, 768], F32, tag="wxTf")
                nc.sync.dma_start(wxT_f[:], wxT)
                wxT_s = xp.tile([64, 768], F32R, tag="wxT")
                nc.vector.tensor_copy(wxT_s[:], wxT_f[:])
                bx_s = xp.tile([128, 6], F32, tag="bx")
                nc.sync.dma_start(bx_s[:], bx)

                nblk = (TN + 511) // 512
                for b in range(BPC):
                    for m in range(6):      # m = gate*2 + chunk
                        g, c = m // 2, m % 2
                        for blk in range(nblk):
                            w = min(512, TN - blk * 512)
                            nt = w // N
                            ps = mpsum.tile([128, 512], F32, tag="mp")
                            nc.tensor.matmul(
                                ps[:, :w],
                                wxT_s[:, m * 128:(m + 1) * 128],
                                xT_s[:, b, blk * 512:blk * 512 + w],
                                start=True, stop=True)
                            xg = xp.tile([128, 8, N], GDT, tag="xg")
                            nc.scalar.activation(
                                xg[:, :nt, :], ps[:, :w].rearrange(
                                    "p (t n) -> p t n", n=N),
                                AF.Identity, bias=bx_s[:, m:m + 1])
                            t0 = blk * 8
                            nc.sync.dma_start(
                                XD[t0:t0 + nt, g, c, :, b * N:(b + 1) * N]
                                .rearrange("t p n -> p t n"),
                                xg[:, :nt, :])

            # ---- main scan over S steps ----
            hid = hidp.tile([128, 2, BN], F32, tag="hidf")
            nc.vector.memset(hid[:], 0.0)
            hidb = hidp.tile([128, 2, BN], GDT, tag="hidb")
            nc.vector.memset(hidb[:], 0.0)

            for t in range(S):
                # X slice prefetch
                xsl = work.tile([128, 3, 2, BN], GDT, tag="xsl")
                nc.sync.dma_start(
                    xsl[:], XD[t].rearrange("g c p n -> p g c n"))

                # node-level W1: Hs/Hr = hidden @ W1{s,r}.T  -> [BN, 256]
                psHs = spsum.tile([128, 384], F32, tag="sp")
                psHr = spsum.tile([128, 384], F32, tag="sp")
                for mat, ps in ((w1sT_s, psHs), (w1rT_s, psHr)):
                    for kc in range(2):
                        nc.tensor.matmul(
                            ps[:, :256], hidb[:, kc, :], mat[:, kc, :],
                            start=(kc == 0), stop=(kc == 1))
                Hs = work.tile([128, 256], EDT, tag="Hs")
                nc.scalar.copy(Hs[:], psHs[:, :256])
                Hr = work.tile([128, 256], EDT, tag="Hr")
                nc.scalar.copy(Hr[:], psHr[:, :256])

                # fc1 gather: msg1[o, e] = relu(Hs[snd] + Hr[rec] + b1)
                # both batch elems concurrently via PE row-tiling (K=64 each)
                m1 = {(bb, c): msgs.tile([128, EG], EDT, tag="msg1",
                                          name=f"m1_{bb}_{c}")
                      for bb in range(BPC) for c in range(2)}
                for c in range(2):
                    cs = slice(c * 128, (c + 1) * 128)
                    for blk in range(8):
                        sl = slice(blk * 512, (blk + 1) * 512)
                        ps0 = mpsum.tile([128, 512], F32, tag="mp")
                        ps1 = mpsum.tile([128, 512], F32, tag="mp")
                        nc.tensor.matmul(ps0, Hs[0:64, cs], rsT_s[0:64, sl],
                                         start=True, stop=False,
                                         tile_position=(0, 0))
                        nc.tensor.matmul(ps1, Hs[64:128, cs], rsT_s[64:128, sl],
                                         start=True, stop=False,
                                         tile_position=(64, 0))
                        nc.tensor.matmul(ps0, Hr[0:64, cs], rrT_s[0:64, sl],
                                         start=False, stop=True,
                                         tile_position=(0, 0))
                        nc.tensor.matmul(ps1, Hr[64:128, cs], rrT_s[64:128, sl],
                                         start=False, stop=True,
                                         tile_position=(64, 0))
                        # relu(+b1) on DVE (frees ACT for relu2)
                        nc.vector.tensor_scalar(
                            m1[(0, c)][:, sl], ps0, b1c_s[:, c:c + 1], 0.0,
                            op0=ALU.add, op1=ALU.max)
                        nc.vector.tensor_scalar(
                            m1[(1, c)][:, sl], ps1, b1c_s[:, c:c + 1], 0.0,
                            op0=ALU.add, op1=ALU.max)

                # fc2 (feature-major) + relu2 (ACT, bias b2) + DMA transpose
                m2e = {}
                for bb in range(BPC):
                    m2 = msgs2.tile([128, 2, EG], EDT, tag="msg2")
                    for mc in range(2):
                        for blk in range(8):
                            sl = slice(blk * 512, (blk + 1) * 512)
                            ps = mpsum.tile([128, 512], F32, tag="mp")
                            nc.tensor.matmul(ps, w2T_s[:, 0, mc, :],
                                             m1[(bb, 0)][:, sl],
                                             start=True, stop=False)
                            nc.tensor.matmul(ps, w2T_s[:, 1, mc, :],
                                             m1[(bb, 1)][:, sl],
                                             start=False, stop=True)
                            nc.scalar.activation(m2[:, mc, sl], ps, AF.Relu,
                                                 bias=b2c_s[:, mc:mc + 1])
                    # edge-major copy via xbar DMA transpose:
                    # chunk k of 512 edges -> [128, 4, 128] (e = 512k + 4p + j)
                    me = msgs2.tile([128, 8, 4, 2, 128], EDT, tag="m2e")
                    for c in range(2):
                        for k in range(8):
                            nc.sync.dma_start_transpose(
                                me[:, k, :, c, :],
                                m2[:, c, k * 512:(k + 1) * 512])
                    m2e[bb] = me

                # edge2node aggregation, both b via PE col-tiling (M=64 each)
                psA = aggps.tile([128, 256], F32, tag="agg")
                for q in range(32):
                    k, j = q // 4, q % 4
                    st, sp = (q == 0), (q == 31)
                    nc.tensor.matmul(psA[0:64, :], wagg_s[:, 0, q, :],
                                     m2e[0][:, k, j, :, :].rearrange(
                                         "p c f -> p (c f)"),
                                     start=st, stop=sp, tile_position=(0, 0))
                    nc.tensor.matmul(psA[64:128, :], wagg_s[:, 1, q, :],
                                     m2e[1][:, k, j, :, :].rearrange(
                                         "p c f -> p (c f)"),
                                     start=st, stop=sp, tile_position=(0, 64))
                aggs = work.tile([128, 256], EDT, tag="aggs")
                nc.scalar.copy(aggs[:], psA[:])
                aggT = work.tile([128, 2, BN], GDT, tag="aggT")
                for c in range(2):
                    nc.sync.dma_start_transpose(
                        aggT[:, c, :], aggs[:, c * 128:(c + 1) * 128])

                # GRU gates (feature-major [256 -> 2 chunks, (b, n)])
                psG = {mc: spsum.tile([128, 384], F32, tag="sp",
                                      name=f"psG_{mc}")
                       for mc in range(2)}
                for mc in range(2):
                    for g in range(3):
                        oap = psG[mc][:, g * 128:(g + 1) * 128]
                        nc.tensor.matmul(oap, whT_s[:, g, 0, mc, :],
                                         aggT[:, 0, :], start=True, stop=False)
                        nc.tensor.matmul(oap, whT_s[:, g, 1, mc, :],
                                         aggT[:, 1, :], start=False,
                                         stop=(g == 2))
                        if g < 2:  # fold x-side into r/i gates on the PE
                            nc.tensor.matmul(oap, ident_s[:], xsl[:, g, mc, :],
                                             start=False, stop=True)
                r_g = work.tile([128, 2, BN], F32, tag="r_g")
                i_g = work.tile([128, 2, BN], F32, tag="i_g")
                hn = work.tile([128, 2, BN], F32, tag="hn")
                for mc in range(2):
                    nc.scalar.activation(r_g[:, mc, :], psG[mc][:, 0:128],
                                         AF.Sigmoid, bias=bh_s[:, 0, mc:mc + 1])
                    nc.scalar.activation(i_g[:, mc, :], psG[mc][:, 128:256],
                                         AF.Sigmoid, bias=bh_s[:, 1, mc:mc + 1])
                    nc.scalar.activation(hn[:, mc, :], psG[mc][:, 256:384],
                                         AF.Identity, bias=bh_s[:, 2, mc:mc + 1])
                # nn = tanh(Xn + r * hn); h' = nn + i * (h - nn)
                tmp = work.tile([128, 2, BN], F32, tag="tmp")
                nc.vector.tensor_mul(tmp[:], r_g[:], hn[:])
                nc.vector.tensor_add(tmp[:], tmp[:], xsl[:, 2, :, :])
                nn = work.tile([128, 2, BN], F32, tag="nn")
                nc.scalar.activation(nn[:], tmp[:], AF.Tanh)
                d = work.tile([128, 2, BN], F32, tag="d")
                nc.vector.tensor_sub(d[:], hid[:], nn[:])
                hid = hidp.tile([128, 2, BN], F32, tag="hidf")
                nc.vector.tensor_mul(d[:], i_g[:], d[:])
                nc.vector.tensor_add(hid[:], nn[:], d[:])
                hidb = hidp.tile([128, 2, BN], GDT, tag="hidb")
                nc.vector.tensor_copy(hidb[:], hid[:])

                # output MLP (feature-major; pred = [do, (b, n)])
                ps1 = spsum.tile([128, 384], F32, tag="sp")
                for mc in range(2):
                    for kc in range(2):
                        nc.tensor.matmul(ps1[:, mc * 128:(mc + 1) * 128],
                                         wo1T_s[:, kc, mc, :], hidb[:, kc, :],
                                         start=(kc == 0), stop=(kc == 1))
                h1 = work.tile([128, 2, BN], GDT, tag="h1")
                for mc in range(2):
                    nc.scalar.activation(h1[:, mc, :],
                                         ps1[:, mc * 128:(mc + 1) * 128],
                                         AF.Relu, bias=bo12_s[:, 0, mc:mc + 1])
                ps2 = spsum.tile([128, 384], F32, tag="sp")
                for mc in range(2):
                    for kc in range(2):
                        nc.tensor.matmul(ps2[:, mc * 128:(mc + 1) * 128],
                                         wo2T_s[:, kc, mc, :], h1[:, kc, :],
                                         start=(kc == 0), stop=(kc == 1))
                h2 = work.tile([128, 2, BN], GDT, tag="h2")
                for mc in range(2):
                    nc.scalar.activation(h2[:, mc, :],
                                         ps2[:, mc * 128:(mc + 1) * 128],
                                         AF.Relu, bias=bo12_s[:, 1, mc:mc + 1])
                ps3 = spsum.tile([64, 384], F32, tag="sp")
                for kc in range(2):
                    nc.tensor.matmul(ps3[:, :BN], wo3T_s[:, kc, :],
                                     h2[:, kc, :],
                                     start=(kc == 0), stop=(kc == 1))
                pred = work.tile([64, BN], F32, tag="pred")
                nc.scalar.activation(pred[:], ps3[:, :BN], AF.Identity,
                                     bias=bo3c_s[:])
                nc.sync.dma_start(
                    predT[t].rearrange("d b n -> d (b n)"), pred[:])

    nc.compile()
    return nc


_CACHE = {}


def _get_program():
    if "nc" not in _CACHE:
        _CACHE["nc"] = build_program()
    return _CACHE["nc"]


def _host_prep(inputs, rel_rec, rel_send, rel_types, weights):
    """Build the per-core in_maps (host-side numpy; value-exact restructure)."""
    x = np.asarray(inputs, np.float32)
    rel_rec = np.asarray(rel_rec, np.float32)
    rel_send = np.asarray(rel_send, np.float32)
    rel_types = np.asarray(rel_types, np.float32)
    w = {k: np.asarray(v, np.float32) for k, v in weights.items()}

    pad = np.zeros((64, EG - E), np.float32)
    rs1 = np.concatenate([rel_send.T, pad], axis=1)          # [64, EG]
    rr1 = np.concatenate([rel_rec.T, pad], axis=1)
    shared = {
        "rsT": np.concatenate([rs1, rs1], 0).astype(NP_EDT),
        "rrT": np.concatenate([rr1, rr1], 0).astype(NP_EDT),
        "w1sT": _chunk2(w["msg_fc1_w"][:, :H].T).astype(NP_EDT),
        "w1rT": _chunk2(w["msg_fc1_w"][:, H:].T).astype(NP_EDT),
        "w2T": _chunk22(w["msg_fc2_w"].T).astype(NP_EDT),
        "whT": np.stack(
            [_chunk22(w[f"gru_h{g}_w"].T) for g in "rin"], axis=1
        ).astype(NP_GDT),
        "ident": np.eye(128).astype(NP_GDT),
        "wo1T": _chunk22(w["out_fc1_w"].T).astype(NP_GDT),
        "wo2T": _chunk22(w["out_fc2_w"].T).astype(NP_GDT),
        "wo3T": _chunk2(w["out_fc3_w"].T).astype(NP_GDT),
        "wxT": np.concatenate(
            [w["gru_ir_w"], w["gru_ii_w"], w["gru_in_w"]], 0).T.copy(),
        "bx": np.concatenate(
            [w["gru_ir_b"], w["gru_ii_b"], w["gru_in_b"]]).reshape(6, 128).T.copy(),
        "b1c": w["msg_fc1_b"].reshape(2, 128).T.copy(),
        "b2c": w["msg_fc2_b"].reshape(2, 128).T.copy(),
        "bh": np.stack([w[f"gru_h{g}_b"].reshape(2, 128).T for g in "rin"],
                       axis=1).copy(),
        "bo12": np.stack([w["out_fc1_b"].reshape(2, 128).T,
                          w["out_fc2_b"].reshape(2, 128).T], axis=1).copy(),
        "bo3c": w["out_fc3_b"][:, None].copy(),
    }
    shared = {k: np.ascontiguousarray(v) for k, v in shared.items()}

    ts = rel_types[:, :, 0] + rel_types[:, :, 1]             # [B, E]
    in_maps = []
    for core in range(NCORES):
        b0 = core * BPC
        xc = x[b0:b0 + BPC, :S]                              # [BPC, S, N, DIN]
        xTc = np.ascontiguousarray(
            xc.transpose(3, 0, 1, 2).reshape(64, BPC, TN).astype(np.float32))
        waggs = []
        for bb in range(BPC):
            wa = ts[b0 + bb][:, None] * rel_rec              # [E, 64]
            wa = np.concatenate([wa, np.zeros((EG - E, 64), np.float32)], 0)
            # permute rows to match DMA-transpose layout: e = 128q + p
            wa = wa.reshape(32, 128, 64).transpose(1, 0, 2)
            waggs.append(wa)
        m = dict(shared)
        m["xT"] = xTc
        m["wagg"] = np.ascontiguousarray(
            np.stack(waggs, axis=1)).astype(NP_EDT)
        in_maps.append(m)
    return in_maps


def kernel(**inputs):
    weights = {k: v for k, v in inputs.items()
               if k not in ("inputs", "rel_rec", "rel_send", "rel_types")}
    in_maps = _host_prep(inputs["inputs"], inputs["rel_rec"],
                         inputs["rel_send"], inputs["rel_types"], weights)
    nc = _get_program()
    res = run_bass_kernel_spmd(nc, in_maps, core_ids=list(range(NCORES)))
    out = np.empty((B, S, N, DOUT), np.float32)
    for core in range(NCORES):
        pt = res.results[core]["predT"]                      # [S, DOUT, BPC, N]
        out[core * BPC:(core + 1) * BPC] = pt.transpose(2, 0, 3, 1)
    return out


# revision 6
# speedup vs baseline: 1.0795x; 1.0795x over previous
"""Trainium2 Bass kernel for nn_GRUDecoder (GNN message-passing GRU decoder).

Sharding: data-parallel over batch B=16 across 8 cores (2 batch elems/core),
all parameters replicated. Forward only -> no collectives.

Math (per step, per batch elem), restructured for TRN2:
  msg1 = relu([senders, receivers] @ W1.T + b1)
       = relu(rel_send @ (hidden @ W1s.T) + rel_rec @ (hidden @ W1r.T) + b1)
    -> W1 applied at NODE level (64 rows), then gathered to 4096 edges via
       one-hot matmuls on the PE (gather + add fused into PSUM accumulation);
       the two batch elems run concurrently via PE row-tiling (K=64 each).
  msg2 = relu(msg1 @ W2.T + b2)            (feature-major, edges stream)
  agg  = (msg2 * type_sum).T @ rel_rec     (type_sum folded into rel_rec
       host-side; msg2 moved to edge-major via xbar DMA transpose; both batch
       elems aggregated concurrently via PE col-tiling, M=64 each)
  GRU + 3-layer output MLP: small feature-major matmuls batched over (b, n);
  x-side GRU projections precomputed for all t in a prologue (float32r).

Everything stays resident in SBUF across the 39 sequential steps.
"""

import sys

for _p in ("/opt/trn_rl_repo", "/opt/pypackages"):
    if _p not in sys.path:
        sys.path.append(_p)

import numpy as np

import concourse.bass as bass
import concourse.mybir as mybir
import concourse.tile as tile
from concourse import bacc
from concourse.bass_utils import run_bass_kernel_spmd

# Problem constants (hardcoded per contract)
B, T, N, DIN = 16, 40, 64, 64
S = T - 1            # 39 scan steps
H = 256
DOUT = 64
E = N * (N - 1)      # 4032
EG = 4096            # edges padded to 4096 (pad rows contribute 0 via W_agg)
NCORES = 8
BPC = B // NCORES    # 2 batch elems per core
BN = BPC * N         # 128 = (b, n) packed partition dim
TN = S * N           # 2496

F32 = mybir.dt.float32
F32R = mybir.dt.float32r
F16 = mybir.dt.float16
EDT = F16            # edge-pipeline matmul dtype
GDT = F16            # gru/output-mlp matmul dtype
NP_EDT = np.float16
NP_GDT = np.float16

AF = mybir.ActivationFunctionType
ALU = mybir.AluOpType


def _chunk2(w):
    """[256, F] -> [128, 2, F] (partition, chunk, free)."""
    f = w.shape[1]
    return np.ascontiguousarray(w.reshape(2, 128, f).transpose(1, 0, 2))


def _chunk22(w):
    """[256, 256] -> [128, kc, mc, 128]."""
    return np.ascontiguousarray(
        w.reshape(2, 128, 2, 128).transpose(1, 0, 2, 3))


def build_program():
    nc = bacc.Bacc("TRN2", target_bir_lowering=False, debug=False)

    def din(name, shape, dtype):
        return nc.dram_tensor(name, list(shape), dtype, kind="ExternalInput").ap()

    xT = din("xT", [64, BPC, TN], F32)            # din-major inputs
    wxT = din("wxT", [64, 768], F32)              # [Wir;Wii;Win].T
    bx = din("bx", [128, 6], F32)                 # x-side (+r/i h-side) biases
    rsT = din("rsT", [128, EG], EDT)              # rel_send.T stacked twice
    rrT = din("rrT", [128, EG], EDT)              # rel_rec.T stacked twice
    w1sT = din("w1sT", [128, 2, 256], EDT)
    w1rT = din("w1rT", [128, 2, 256], EDT)
    w2T = din("w2T", [128, 2, 2, 128], EDT)
    wagg = din("wagg", [128, BPC, 32, 64], EDT)   # ts-scaled rel_rec, permuted
    whT = din("whT", [128, 3, 2, 2, 128], GDT)    # gru_h{r,i,n}_w.T
    ident = din("ident", [128, 128], GDT)
    wo1T = din("wo1T", [128, 2, 2, 128], GDT)
    wo2T = din("wo2T", [128, 2, 2, 128], GDT)
    wo3T = din("wo3T", [128, 2, 64], GDT)
    b1c = din("b1c", [128, 2], F32)
    b2c = din("b2c", [128, 2], F32)
    bhn = din("bhn", [128, 2], F32)               # n-gate h-side bias
    bo12 = din("bo12", [128, 2, 2], F32)
    bo3c = din("bo3c", [64, 1], F32)

    predT = nc.dram_tensor("predT", [S, DOUT, BPC, N], F32,
                           kind="ExternalOutput").ap()

    with tile.TileContext(nc) as tc:
        with (
            tc.tile_pool(name="const", bufs=1) as const,
            tc.tile_pool(name="mpsum", bufs=2, space="PSUM") as mpsum,
            tc.tile_pool(name="spsum", bufs=4, space="PSUM") as spsum,
            tc.tile_pool(name="msgs", bufs=4) as msgs,
            tc.tile_pool(name="msgs2", bufs=2) as msgs2,
            tc.tile_pool(name="work", bufs=2) as work,
            tc.tile_pool(name="hidp", bufs=2) as hidp,
            tc.tile_pool(name="xdram", bufs=1, space="DRAM") as xdram,
        ):
            # ---- load constants into SBUF ----
            def cload(ap_in, shape, dtype, tag):
                t = const.tile(list(shape), dtype, tag=tag, name=tag)
                nc.sync.dma_start(t[:], ap_in)
                return t

            rsT_s = cload(rsT, [128, EG], EDT, "rsT")
            rrT_s = cload(rrT, [128, EG], EDT, "rrT")
            w1sT_s = cload(w1sT, [128, 2, 256], EDT, "w1sT")
            w1rT_s = cload(w1rT, [128, 2, 256], EDT, "w1rT")
            w2T_s = cload(w2T, [128, 2, 2, 128], EDT, "w2T")
            wagg_s = cload(wagg, [128, BPC, 32, 64], EDT, "wagg")
            whT_s = cload(whT, [128, 3, 2, 2, 128], GDT, "whT")
            ident_s = cload(ident, [128, 128], GDT, "ident")
            wo1T_s = cload(wo1T, [128, 2, 2, 128], GDT, "wo1T")
            wo2T_s = cload(wo2T, [128, 2, 2, 128], GDT, "wo2T")
            wo3T_s = cload(wo3T, [128, 2, 64], GDT, "wo3T")
            b1c_s = cload(b1c, [128, 2], F32, "b1c")
            b2c_s = cload(b2c, [128, 2], F32, "b2c")
            bhn_s = cload(bhn, [128, 2], F32, "bhn")
            bo12_s = cload(bo12, [128, 2, 2], F32, "bo12")
            bo3c_s = cload(bo3c, [64, 1], F32, "bo3c")

            # X-gate DRAM scratch: [S, gate, chunk, go_part, (b, n)]
            XD = xdram.tile([S, 3, 2, 128, BN], GDT)

            # ---- phase X: precompute x-side GRU gate inputs for all t ----
            with tc.tile_pool(name="xphase", bufs=1) as xp:
                xT_f = xp.tile([64, BPC, TN], F32, tag="xTf")
                nc.sync.dma_start(xT_f[:], xT)
                xT_s = xp.tile([64, BPC, TN], F32R, tag="xT")
                nc.vector.tensor_copy(xT_s[:], xT_f[:])
                wxT_f = xp.tile([64, 768], F32, tag="wxTf")
                nc.sync.dma_start(wxT_f[:], wxT)
                wxT_s = xp.tile([64, 768], F32R, tag="wxT")
                nc.vector.tensor_copy(wxT_s[:], wxT_f[:])
                bx_s = xp.tile([128, 6], F32, tag="bx")
                nc.sync.dma_start(bx_s[:], bx)

                nblk = (TN + 511) // 512
                for b in range(BPC):
                    for m in range(6):      # m = gate*2 + chunk
                        g, c = m // 2, m % 2
                        for blk in range(nblk):
                            w = min(512, TN - blk * 512)
                            nt = w // N
                            ps = mpsum.tile([128, 1024], F32, tag="mp",
                                            name="psx")
                            nc.tensor.matmul(
                                ps[:, :w],
                                wxT_s[:, m * 128:(m + 1) * 128],
                                xT_s[:, b, blk * 512:blk * 512 + w],
                                start=True, stop=True)
                            xg = xp.tile([128, 8, N], GDT, tag="xg")
                            nc.scalar.activation(
                                xg[:, :nt, :], ps[:, :w].rearrange(
                                    "p (t n) -> p t n", n=N),
                                AF.Identity, bias=bx_s[:, m:m + 1])
                            t0 = blk * 8
                            nc.sync.dma_start(
                                XD[t0:t0 + nt, g, c, :, b * N:(b + 1) * N]
                                .rearrange("t p n -> p t n"),
                                xg[:, :nt, :])

            # ---- main scan over S steps ----
            hid = hidp.tile([128, 2, BN], F32, tag="hidf")
            nc.vector.memset(hid[:], 0.0)
            hidb = hidp.tile([128, 2, BN], GDT, tag="hidb")
            nc.vector.memset(hidb[:], 0.0)

            for t in range(S):
                # X slice prefetch
                xsl = work.tile([128, 3, 2, BN], GDT, tag="xsl")
                nc.sync.dma_start(
                    xsl[:], XD[t].rearrange("g c p n -> p g c n"))

                # node-level W1: Hs/Hr = hidden @ W1{s,r}.T  -> [(b,n), 256]
                psHs = spsum.tile([128, 384], F32, tag="sp")
                psHr = spsum.tile([128, 384], F32, tag="sp")
                for mat, ps in ((w1sT_s, psHs), (w1rT_s, psHr)):
                    for kc in range(2):
                        nc.tensor.matmul(
                            ps[:, :256], hidb[:, kc, :], mat[:, kc, :],
                            start=(kc == 0), stop=(kc == 1))
                Hs = work.tile([128, 256], EDT, tag="Hs")
                nc.scalar.copy(Hs[:], psHs[:, :256])
                Hr = work.tile([128, 256], EDT, tag="Hr")
                nc.vector.tensor_copy(Hr[:], psHr[:, :256])

                # fc1 gather: msg1[o, e] = relu(Hs[snd] + Hr[rec] + b1)
                # both batch elems concurrently via PE row-tiling (K=64 each)
                m1 = {(bb, c): msgs.tile([128, EG], EDT, tag="msg1",
                                         name=f"m1_{bb}_{c}")
                      for bb in range(BPC) for c in range(2)}
                for c in range(2):
                    cs = slice(c * 128, (c + 1) * 128)
                    for b2 in range(4):
                        ps0 = mpsum.tile([128, 1024], F32, tag="mp",
                                         name="ps0")
                        ps1 = mpsum.tile([128, 1024], F32, tag="mp",
                                         name="ps1")
                        for hf in range(2):
                            sl = slice(b2 * 1024 + hf * 512,
                                       b2 * 1024 + (hf + 1) * 512)
                            ph = slice(hf * 512, (hf + 1) * 512)
                            nc.tensor.matmul(ps0[:, ph], Hs[0:64, cs],
                                             rsT_s[0:64, sl],
                                             start=True, stop=False,
                                             tile_position=(0, 0))
                            nc.tensor.matmul(ps1[:, ph], Hs[64:128, cs],
                                             rsT_s[64:128, sl],
                                             start=True, stop=False,
                                             tile_position=(64, 0))
                            nc.tensor.matmul(ps0[:, ph], Hr[0:64, cs],
                                             rrT_s[0:64, sl],
                                             start=False, stop=True,
                                             tile_position=(0, 0))
                            nc.tensor.matmul(ps1[:, ph], Hr[64:128, cs],
                                             rrT_s[64:128, sl],
                                             start=False, stop=True,
                                             tile_position=(64, 0))
                        bsl = slice(b2 * 1024, (b2 + 1) * 1024)
                        # relu(+b1) on DVE (frees ACT for relu2)
                        nc.vector.tensor_scalar(
                            m1[(0, c)][:, bsl], ps0, b1c_s[:, c:c + 1], 0.0,
                            op0=ALU.add, op1=ALU.max)
                        nc.vector.tensor_scalar(
                            m1[(1, c)][:, bsl], ps1, b1c_s[:, c:c + 1], 0.0,
                            op0=ALU.add, op1=ALU.max)

                # fc2 (feature-major) + relu2 (ACT, bias b2) + DMA transpose
                m2e = {}
                for bb in range(BPC):
                    m2 = msgs2.tile([128, 2, EG], EDT, tag="msg2", name="m2")
                    for mc in range(2):
                        for b2 in range(4):
                            ps = mpsum.tile([128, 1024], F32, tag="mp",
                                            name="psf")
                            for hf in range(2):
                                sl = slice(b2 * 1024 + hf * 512,
                                           b2 * 1024 + (hf + 1) * 512)
                                ph = slice(hf * 512, (hf + 1) * 512)
                                nc.tensor.matmul(ps[:, ph], w2T_s[:, 0, mc, :],
                                                 m1[(bb, 0)][:, sl],
                                                 start=True, stop=False)
                                nc.tensor.matmul(ps[:, ph], w2T_s[:, 1, mc, :],
                                                 m1[(bb, 1)][:, sl],
                                                 start=False, stop=True)
                            bsl = slice(b2 * 1024, (b2 + 1) * 1024)
                            nc.scalar.activation(m2[:, mc, bsl], ps, AF.Relu,
                                                 bias=b2c_s[:, mc:mc + 1])
                    # edge-major copy via xbar DMA transpose:
                    # transpose block k covers edges e = 512k + 128j + p
                    me = msgs2.tile([128, 8, 4, 2, 128], EDT, tag="m2e",
                                    name="me")
                    for c in range(2):
                        for k in range(8):
                            nc.sync.dma_start_transpose(
                                me[:, k, :, c, :],
                                m2[:, c, k * 512:(k + 1) * 512])
                    m2e[bb] = me

                # edge2node aggregation, both b via PE col-tiling (M=64 each)
                psA = spsum.tile([128, 384], F32, tag="sp", name="psA")
                for q in range(32):
                    k, j = q // 4, q % 4
                    st, sp = (q == 0), (q == 31)
                    nc.tensor.matmul(psA[0:64, :256], wagg_s[:, 0, q, :],
                                     m2e[0][:, k, j, :, :].rearrange(
                                         "p c f -> p (c f)"),
                                     start=st, stop=sp, tile_position=(0, 0))
                    nc.tensor.matmul(psA[64:128, :256], wagg_s[:, 1, q, :],
                                     m2e[1][:, k, j, :, :].rearrange(
                                         "p c f -> p (c f)"),
                                     start=st, stop=sp, tile_position=(0, 64))
                aggs = work.tile([128, 256], EDT, tag="aggs")
                nc.scalar.copy(aggs[:], psA[:, :256])
                # transpose agg to feature-major on the PE (short critical path)
                aggT = work.tile([128, 2, BN], GDT, tag="aggT")
                for c in range(2):
                    pt = spsum.tile([128, 128], GDT, tag="sp",
                                    name=f"pt{c}")
                    nc.tensor.transpose(pt[:], aggs[:, c * 128:(c + 1) * 128],
                                        ident_s[:])
                    if c == 0:
                        nc.scalar.copy(aggT[:, c, :], pt[:])
                    else:
                        nc.vector.tensor_copy(aggT[:, c, :], pt[:])

                # GRU gates (feature-major [256 -> 2 chunks, (b, n)])
                # r/i biases (x-side + h-side) pre-folded into X; bhn separate
                psR = spsum.tile([128, 256], F32, tag="sp", name="psR")
                psI = spsum.tile([128, 256], F32, tag="sp", name="psI")
                psN = spsum.tile([128, 256], F32, tag="sp", name="psN")
                for g, psg in ((0, psR), (1, psI), (2, psN)):
                    for mc in range(2):
                        oap = psg[:, mc * 128:(mc + 1) * 128]
                        nc.tensor.matmul(oap, whT_s[:, g, 0, mc, :],
                                         aggT[:, 0, :], start=True, stop=False)
                        nc.tensor.matmul(oap, whT_s[:, g, 1, mc, :],
                                         aggT[:, 1, :], start=False,
                                         stop=(g == 2))
                        if g < 2:  # fold x-side (+biases) into r/i on the PE
                            nc.tensor.matmul(oap, ident_s[:], xsl[:, g, mc, :],
                                             start=False, stop=True)
                r_g = work.tile([128, 2, BN], F32, tag="r_g")
                nc.scalar.activation(
                    r_g[:].rearrange("p c n -> p (c n)"), psR[:], AF.Sigmoid)
                i_g = work.tile([128, 2, BN], F32, tag="i_g")
                nc.scalar.activation(
                    i_g[:].rearrange("p c n -> p (c n)"), psI[:], AF.Sigmoid)
                # tmp = (hn_psum + bhn) * r   (fused on DVE, psum source)
                tmp = work.tile([128, 2, BN], F32, tag="tmp")
                for mc in range(2):
                    nc.vector.scalar_tensor_tensor(
                        tmp[:, mc, :], psN[:, mc * 128:(mc + 1) * 128],
                        bhn_s[:, mc:mc + 1], r_g[:, mc, :],
                        op0=ALU.add, op1=ALU.mult)
                nc.vector.tensor_add(tmp[:], tmp[:], xsl[:, 2, :, :])
                nn = work.tile([128, 2, BN], F32, tag="nn")
                nc.scalar.activation(
                    nn[:].rearrange("p c n -> p (c n)"),
                    tmp[:].rearrange("p c n -> p (c n)"), AF.Tanh)
                # h' = nn + i * (h - nn)
                d = work.tile([128, 2, BN], F32, tag="d")
                nc.vector.tensor_sub(d[:], hid[:], nn[:])
                hid = hidp.tile([128, 2, BN], F32, tag="hidf")
                nc.vector.tensor_mul(d[:], i_g[:], d[:])
                nc.vector.tensor_add(hid[:], nn[:], d[:])
                hidb = hidp.tile([128, 2, BN], GDT, tag="hidb")
                nc.vector.tensor_copy(hidb[:], hid[:])

                # output MLP (feature-major; pred = [do, (b, n)])
                ps1 = spsum.tile([128, 384], F32, tag="sp", name="po1")
                for mc in range(2):
                    for kc in range(2):
                        nc.tensor.matmul(ps1[:, mc * 128:(mc + 1) * 128],
                                         wo1T_s[:, kc, mc, :], hidb[:, kc, :],
                                         start=(kc == 0), stop=(kc == 1))
                h1 = work.tile([128, 2, BN], GDT, tag="h1")
                for mc in range(2):
                    nc.scalar.activation(h1[:, mc, :],
                                         ps1[:, mc * 128:(mc + 1) * 128],
                                         AF.Relu, bias=bo12_s[:, 0, mc:mc + 1])
                ps2 = spsum.tile([128, 384], F32, tag="sp", name="po2")
                for mc in range(2):
                    for kc in range(2):
                        nc.tensor.matmul(ps2[:, mc * 128:(mc + 1) * 128],
                                         wo2T_s[:, kc, mc, :], h1[:, kc, :],
                                         start=(kc == 0), stop=(kc == 1))
                h2 = work.tile([128, 2, BN], GDT, tag="h2")
                for mc in range(2):
                    nc.scalar.activation(h2[:, mc, :],
                                         ps2[:, mc * 128:(mc + 1) * 128],
                                         AF.Relu, bias=bo12_s[:, 1, mc:mc + 1])
                ps3 = spsum.tile([64, 384], F32, tag="sp", name="po3")
                for kc in range(2):
                    nc.tensor.matmul(ps3[:, :BN], wo3T_s[:, kc, :],
                                     h2[:, kc, :],
                                     start=(kc == 0), stop=(kc == 1))
                pred = work.tile([64, BN], F32, tag="pred")
                nc.scalar.activation(pred[:], ps3[:, :BN], AF.Identity,
                                     bias=bo3c_s[:])
                nc.sync.dma_start(
                    predT[t].rearrange("d b n -> d (b n)"), pred[:])

    nc.compile()
    return nc


_CACHE = {}


def _get_program():
    if "nc" not in _CACHE:
        _CACHE["nc"] = build_program()
    return _CACHE["nc"]


def _host_prep(inputs, rel_rec, rel_send, rel_types, weights):
    """Build the per-core in_maps (host-side numpy; value-exact restructure)."""
    x = np.asarray(inputs, np.float32)
    rel_rec = np.asarray(rel_rec, np.float32)
    rel_send = np.asarray(rel_send, np.float32)
    rel_types = np.asarray(rel_types, np.float32)
    w = {k: np.asarray(v, np.float32) for k, v in weights.items()}

    pad = np.zeros((64, EG - E), np.float32)
    rs1 = np.concatenate([rel_send.T, pad], axis=1)          # [64, EG]
    rr1 = np.concatenate([rel_rec.T, pad], axis=1)
    # x-side biases + r/i h-side biases (folded so gate sigmoids need no bias)
    bx_np = np.concatenate([w["gru_ir_b"] + w["gru_hr_b"],
                            w["gru_ii_b"] + w["gru_hi_b"],
                            w["gru_in_b"]]).reshape(6, 128).T
    shared = {
        "rsT": np.concatenate([rs1, rs1], 0).astype(NP_EDT),
        "rrT": np.concatenate([rr1, rr1], 0).astype(NP_EDT),
        "w1sT": _chunk2(w["msg_fc1_w"][:, :H].T).astype(NP_EDT),
        "w1rT": _chunk2(w["msg_fc1_w"][:, H:].T).astype(NP_EDT),
        "w2T": _chunk22(w["msg_fc2_w"].T).astype(NP_EDT),
        "whT": np.stack(
            [_chunk22(w[f"gru_h{g}_w"].T) for g in "rin"], axis=1
        ).astype(NP_GDT),
        "ident": np.eye(128).astype(NP_GDT),
        "wo1T": _chunk22(w["out_fc1_w"].T).astype(NP_GDT),
        "wo2T": _chunk22(w["out_fc2_w"].T).astype(NP_GDT),
        "wo3T": _chunk2(w["out_fc3_w"].T).astype(NP_GDT),
        "wxT": np.concatenate(
            [w["gru_ir_w"], w["gru_ii_w"], w["gru_in_w"]], 0).T.copy(),
        "bx": bx_np.copy(),
        "b1c": w["msg_fc1_b"].reshape(2, 128).T.copy(),
        "b2c": w["msg_fc2_b"].reshape(2, 128).T.copy(),
        "bhn": w["gru_hn_b"].reshape(2, 128).T.copy(),
        "bo12": np.stack([w["out_fc1_b"].reshape(2, 128).T,
                          w["out_fc2_b"].reshape(2, 128).T], axis=1).copy(),
        "bo3c": w["out_fc3_b"][:, None].copy(),
    }
    shared = {k: np.ascontiguousarray(v) for k, v in shared.items()}

    ts = rel_types[:, :, 0] + rel_types[:, :, 1]             # [B, E]
    in_maps = []
    for core in range(NCORES):
        b0 = core * BPC
        xc = x[b0:b0 + BPC, :S]                              # [BPC, S, N, DIN]
        xTc = np.ascontiguousarray(
            xc.transpose(3, 0, 1, 2).reshape(64, BPC, TN).astype(np.float32))
        waggs = []
        for bb in range(BPC):
            wa = ts[b0 + bb][:, None] * rel_rec              # [E, 64]
            wa = np.concatenate([wa, np.zeros((EG - E, 64), np.float32)], 0)
            # rows permuted to match DMA-transpose layout: e = 128q + p
            wa = wa.reshape(32, 128, 64).transpose(1, 0, 2)
            waggs.append(wa)
        m = dict(shared)
        m["xT"] = xTc
        m["wagg"] = np.ascontiguousarray(
            np.stack(waggs, axis=1)).astype(NP_EDT)
        in_maps.append(m)
    return in_maps


def kernel(**inputs):
    weights = {k: v for k, v in inputs.items()
               if k not in ("inputs", "rel_rec", "rel_send", "rel_types")}
    in_maps = _host_prep(inputs["inputs"], inputs["rel_rec"],
                         inputs["rel_send"], inputs["rel_types"], weights)
    nc = _get_program()
    res = run_bass_kernel_spmd(nc, in_maps, core_ids=list(range(NCORES)))
    out = np.empty((B, S, N, DOUT), np.float32)
    for core in range(NCORES):
        pt = res.results[core]["predT"]                      # [S, DOUT, BPC, N]
        out[core * BPC:(core + 1) * BPC] = pt.transpose(2, 0, 3, 1)
    return out
